# revision 29
# baseline (speedup 1.0000x reference)
"""Trainium2 Bass kernel for PVT-style spatial-reduction attention.

Model (see reference):
  q = (x @ Wq + bq) * hd^-0.5                       (B, N, C) -> heads of 32
  x_ = BN(DWConv2x2s2(x)) ; k = x_ @ Wk + bk ; v = x_ @ Wv + bv
  attn = softmax(q k^T + rel_pos) ; out = (attn @ v) @ Wp + bp

Shapes: B=8, N=3136 (56x56), C=128, heads=4, hd=32, Nkv=784 (28x28).

Distribution: each of 8 cores handles a slice of 392 query rows (N/8) for
ALL batches and heads.  rel_pos then splits exactly 8 ways and each core
produces final output rows locally (no cross-core reduction).

Device layout strategy: features-on-partitions everywhere (C == 128).
  - host passes xT (B, C, N) in bf16; all projections are lhsT=weight
    matmuls.
  - conv+BN+k/v projection fused into 4 "tap" weight matrices (host
    precomputed); k-bias dropped (softmax-invariant), v-bias folded into
    the final bias.
  - scores computed transposed: S^T[m, n] per (b, h); softmax uses
    exp(S + R) = exp(S) * exp(R) with exp(rel_pos^T) precomputed on host.
    No max-subtraction (|S| < 1 by construction).
  - per (batch, kv-chunk): attn@v runs all 4 heads 4-way column-packed
    (32-wide) into ONE psum bank; row sums via 4 column-packed ones-vector
    matmuls into a second bank.  Extraction is one [128, 392] copy.
  - normalization: rowsums -> 4 partitions (DMA gather) -> block-broadcast
    matmul -> reciprocal_approx_fast -> multiply (bf16).
  - engine balance: ScalarE does exp only; tap-projection PSUM->SBUF
    copies run on VectorE; 5 of 14 exp(R) multiplies per batch run on the
    otherwise-idle GpSimd engine.
  - software pipeline per batch b (14 half-round steps):
      all steps:   scores+exp+mul of b (chunk step//2, head-pair step%2)
      even steps:  attn@v + rowsums of b-1, one kv chunk per step
      step 13:     extract of b-1
      steps 0-1:   normalize + projection tail of b-2
      steps 2-13:  prep of b+1 (q, taps, v-transposes)
  - final output is produced transposed (B, C, NSL); the host gather
    untransposes while assembling the full (B, N, C) result.
"""

import os
import sys

import numpy as np

if "/opt/trn_rl_repo" not in sys.path:
    sys.path.insert(0, "/opt/trn_rl_repo")

B = 8
N = 3136
C = 128
HEADS = 4
HD = 32
SR = 2
H = W = 56
NKV = 784  # 28*28
NCORES = 8
NSL = N // NCORES  # 392 query rows per core
BN_EPS = 1e-5
SCALE = HD ** -0.5

# m (kv index) chunking: 784 = 6*128 + 16
M_CHUNKS = [(j * 128, min(128, NKV - j * 128)) for j in range((NKV + 127) // 128)]

PROB_BF16 = os.environ.get("KERNEL_PROB_BF16", "1") == "1"
# (r, hp) half-rounds whose exp(R) multiply runs on GpSimd instead of DVE
GPS_MULS = {(r, 1) for r in range(5)}

_COMPILED = None  # cached nc across kernel() calls


def _host_prep(x, relative_pos, Wq, bq, Wk, bk, Wv, bv, conv_w, conv_b,
               bn_gamma, bn_beta, bn_mean, bn_var, Wp, bp):
    """Fuse conv/BN into tap weights; fold biases; transpose activations."""
    import ml_dtypes
    f32 = np.float32
    bf16 = ml_dtypes.bfloat16
    wdt = bf16 if PROB_BF16 else f32
    x = np.asarray(x, f32)
    # xT: (B, C, N)
    xT = np.ascontiguousarray(x.transpose(0, 2, 1).astype(wdt))

    inv = (np.asarray(bn_gamma, f32)
           / np.sqrt(np.asarray(bn_var, f32) + BN_EPS))          # [c]
    wp_taps = np.asarray(conv_w, f32).reshape(C, SR * SR) * inv[:, None]  # [c,4]
    beta0 = (np.asarray(conv_b, f32) * inv
             + np.asarray(bn_beta, f32)
             - np.asarray(bn_mean, f32) * inv)                    # [c]

    Wk = np.asarray(Wk, f32)
    Wv = np.asarray(Wv, f32)
    # Wk_tap[t, c, c'] = wp_taps[c, t] * Wk[c, c']
    Wk_tap = np.ascontiguousarray(
        (wp_taps.T[:, :, None] * Wk[None, :, :]).astype(wdt))     # (4, C, C)
    Wv_tap = np.ascontiguousarray(
        (wp_taps.T[:, :, None] * Wv[None, :, :]).astype(wdt))

    # v bias (uniform over kv positions -> exact fold into final bias)
    beta_v = beta0 @ Wv + np.asarray(bv, f32)                     # [c']
    bp_col = (np.asarray(bp, f32) + beta_v @ np.asarray(Wp, f32)).reshape(C, 1)

    Wq_s = np.ascontiguousarray((np.asarray(Wq, f32) * SCALE).astype(wdt))
    bq_col = (np.asarray(bq, f32) * SCALE).reshape(C, 1)

    # exp(rel)^T per core: (4, NKV, NSL)
    rel = np.asarray(relative_pos, f32)
    expRT = []
    for j in range(NCORES):
        sl = rel[:, j * NSL:(j + 1) * NSL, :]          # (4, NSL, NKV)
        e = np.exp(sl).transpose(0, 2, 1)              # (4, NKV, NSL)
        if PROB_BF16:
            e = e.astype(bf16)
        expRT.append(np.ascontiguousarray(e))

    return dict(xT=xT, Wk_tap=Wk_tap, Wv_tap=Wv_tap, Wq=Wq_s, bq=bq_col,
                Wp=np.ascontiguousarray(np.asarray(Wp, f32).astype(wdt)),
                bp=bp_col, expRT=expRT)


def _build():
    """Build + compile the SPMD bass program (same NEFF for all 8 cores)."""
    import concourse.bass as bass
    import concourse.tile as tile
    from concourse import bacc, mybir
    from concourse.masks import make_identity

    f32 = mybir.dt.float32
    f32r = mybir.dt.float32r
    pdt = mybir.dt.bfloat16 if PROB_BF16 else f32

    nc = bacc.Bacc("TRN2", target_bir_lowering=False, debug=False,
                   num_devices=NCORES)

    # ---- DRAM I/O ----
    xT_d = nc.dram_tensor("xT", [B, C, N], pdt, kind="ExternalInput").ap()
    xTn_d = nc.dram_tensor("xTn", [B, C, NSL], pdt, kind="ExternalInput").ap()
    expRT_d = nc.dram_tensor("expRT", [HEADS, NKV, NSL],
                             pdt, kind="ExternalInput").ap()
    Wq_d = nc.dram_tensor("Wq", [C, C], pdt, kind="ExternalInput").ap()
    bq_d = nc.dram_tensor("bq", [C, 1], f32, kind="ExternalInput").ap()
    Wktap_d = nc.dram_tensor("Wktap", [SR * SR, C, C], pdt,
                             kind="ExternalInput").ap()
    Wvtap_d = nc.dram_tensor("Wvtap", [SR * SR, C, C], pdt,
                             kind="ExternalInput").ap()
    Wp_d = nc.dram_tensor("Wp", [C, C], pdt, kind="ExternalInput").ap()
    bp_d = nc.dram_tensor("bp", [C, 1], f32, kind="ExternalInput").ap()
    out_d = nc.dram_tensor("out", [B, C, NSL], f32, kind="ExternalOutput").ap()

    with tile.TileContext(nc) as tc:
        from contextlib import ExitStack
        with ExitStack() as ctx:
            _emit(ctx, tc, nc, bass, mybir, make_identity, f32, f32r, pdt,
                  xT_d, xTn_d, expRT_d, Wq_d, bq_d, Wktap_d, Wvtap_d,
                  Wp_d, bp_d, out_d)

    nc.compile()
    return nc


def _emit(ctx, tc, nc, bass, mybir, make_identity, f32, f32r, pdt,
          xT_d, xTn_d, expRT_d, Wq_d, bq_d, Wktap_d, Wvtap_d,
          Wp_d, bp_d, out_d):
    AF = mybir.ActivationFunctionType

    singles = ctx.enter_context(tc.tile_pool(name="singles", bufs=1))
    xpool = ctx.enter_context(tc.tile_pool(name="xpool", bufs=3))
    qkv = ctx.enter_context(tc.tile_pool(name="qkv", bufs=3))
    ppool = ctx.enter_context(tc.tile_pool(name="ppool", bufs=3))
    opool = ctx.enter_context(tc.tile_pool(name="opool", bufs=3))
    vpool = ctx.enter_context(tc.tile_pool(name="vpool", bufs=3))
    ptpool = ctx.enter_context(tc.tile_pool(name="ptpool", bufs=6))
    # PSUM: rot 3x2 + out 1 + rs 1 = 8 banks.  One deep rotation shared by
    # scores and all transient prep tiles: scores get ~3-slot slack vs exp,
    # so the PE can run ahead and ScalarE stays saturated.
    ps_rot = ctx.enter_context(tc.tile_pool(name="ps_rot", bufs=3,
                                            space="PSUM"))
    ps_out = ctx.enter_context(tc.tile_pool(name="ps_out", bufs=1,
                                            space="PSUM"))
    ps_rs = ctx.enter_context(tc.tile_pool(name="ps_rs", bufs=1,
                                           space="PSUM"))

    # ---- constants ----
    identb = singles.tile([C, C], pdt)
    make_identity(nc, identb[:])
    ones_sb = singles.tile([C, HD], pdt)
    nc.vector.memset(ones_sb[:], 1.0)

    wq_sb = singles.tile([C, C], pdt)
    nc.sync.dma_start(out=wq_sb[:], in_=Wq_d)
    bq_sb = singles.tile([C, 1], f32)
    nc.sync.dma_start(out=bq_sb[:], in_=bq_d)
    wk_sb = singles.tile([C, SR * SR, C], pdt)
    nc.sync.dma_start(out=wk_sb[:], in_=Wktap_d.rearrange("t c d -> c t d"))
    wv_sb = singles.tile([C, SR * SR, C], pdt)
    nc.sync.dma_start(out=wv_sb[:], in_=Wvtap_d.rearrange("t c d -> c t d"))
    wp_sb = singles.tile([C, C], pdt)
    nc.sync.dma_start(out=wp_sb[:], in_=Wp_d)
    bp_sb = singles.tile([C, 1], f32)
    nc.sync.dma_start(out=bp_sb[:], in_=bp_d)

    # expRT interleaved: [128, 7 chunks, 4 heads, 392]
    expTI = singles.tile([C, 7, HEADS, NSL], pdt)
    nc.vector.memset(expTI[:, 6, :, :], 0.0)
    for h in range(HEADS):
        src = expRT_d[h]  # (784, 392)
        nc.sync.dma_start(
            out=expTI[:, 0:6, h, :],
            in_=src[0:768].rearrange("(j p) i -> p j i", p=128))
        nc.sync.dma_start(out=expTI[0:16, 6, h, :], in_=src[768:784])

    state = {}
    pp_of = {}

    def prep_load(b):
        s = state.setdefault(b, {})
        xT_sb = xpool.tile([C, N], pdt, tag="xT")
        s["xT"] = xT_sb
        nc.sync.dma_start(out=xT_sb[:, 0:N // 2], in_=xT_d[b, :, 0:N // 2])
        nc.sync.dma_start(out=xT_sb[:, N // 2:N], in_=xT_d[b, :, N // 2:N])
        xTn_sb = xpool.tile([C, NSL], pdt, tag="xTn")
        s["xTn"] = xTn_sb
        nc.sync.dma_start(out=xTn_sb[:], in_=xTn_d[b])

    def prep_q(b):
        s = state[b]
        ps_q = ps_rot.tile([C, 2, 512], f32, tag="rot", name="ps_q")
        ps_q = ps_q[:, 0, :]
        nc.tensor.matmul(ps_q[:, 0:NSL], lhsT=wq_sb[:], rhs=s.pop("xTn")[:],
                         start=True, stop=True)
        qT_sb = qkv.tile([C, NSL], pdt, tag="qT")
        s["qT"] = qT_sb
        nc.vector.tensor_scalar_add(qT_sb[:], ps_q[:, 0:NSL], bq_sb[:, 0:1])
        kT_sb = qkv.tile([C, 7 * 128], pdt, tag="kT")
        s["kT"] = kT_sb
        nc.vector.memset(kT_sb[:, NKV:7 * 128], 0.0)
        vT_sb = qkv.tile([C, NKV], pdt, tag="vT")
        s["vT"] = vT_sb

    def prep_tap(b, which, mc):
        """One kv-chunk of the fused conv-tap projection (4 matmuls)."""
        s = state[b]
        dst = s["kT"] if which == 0 else s["vT"]
        w_sb = wk_sb if which == 0 else wv_sb
        xview = s["xT"][:].rearrange("p (i a j c) -> p a c i j",
                                     i=28, a=2, j=28, c=2)
        ps_kv = ps_rot.tile([C, 2, 512], f32, tag="rot", name="ps_kv")
        ps_kv = ps_kv[:, 0, :]
        for t in range(SR * SR):
            di, dj = t // 2, t % 2
            rhs = xview[:, di, dj, 14 * mc:14 * mc + 14, :]
            nc.tensor.matmul(ps_kv[:, 0:392], lhsT=w_sb[:, t, :],
                             rhs=rhs, start=(t == 0), stop=(t == 3))
        nc.vector.tensor_copy(dst[:, 392 * mc:392 * (mc + 1)], ps_kv[:, 0:392])

    def prep_valloc(b):
        s = state[b]
        s["v"] = vpool.tile([C, 7, HEADS, HD], pdt, tag="v", name="v_sb")

    def prep_vtrans(b, j):
        s = state[b]
        m0, cnt = M_CHUNKS[j]
        ps_t = ps_rot.tile([C, 2, 512], pdt, tag="rot", name="ps_t")
        ps_t = ps_t[:, 0, :]
        nc.tensor.transpose(ps_t[0:cnt, 0:C], s["vT"][:, m0:m0 + cnt],
                            identb[:])
        nc.vector.tensor_copy(
            s["v"][0:cnt, j, :, :],
            ps_t[0:cnt, 0:C].rearrange("p (h d) -> p h d", h=HEADS, d=HD))

    def half_round(b, r, hp):
        """Scores + exp + expR multiply for chunk r, head pair hp."""
        s = state[b]
        ps_s = ps_rot.tile([C, 2, 512], f32, tag="rot", name="ps_s")
        for hh in range(2):
            h = 2 * hp + hh
            nc.tensor.matmul(
                ps_s[0:128, hh, 0:NSL],
                lhsT=s["kT"][HD * h:HD * (h + 1), 128 * r:128 * (r + 1)],
                rhs=s["qT"][HD * h:HD * (h + 1), :],
                start=True, stop=True,
                tile_position=(HD * h, 0))
        pt_sb = ptpool.tile([C, 2, NSL], pdt, tag="pt")
        nc.scalar.activation(pt_sb[:], ps_s[:, :, 0:NSL], AF.Exp)
        eng = nc.gpsimd if (r, hp) in GPS_MULS else nc.vector
        eng.tensor_mul(pp_of[b][:, r, 2 * hp:2 * hp + 2, :], pt_sb[:],
                       expTI[:, r, 2 * hp:2 * hp + 2, :])

    def attnv4(b, r):
        """attn@v + rowsums for kv chunk r: all 4 heads column-packed."""
        s = state[b]
        m0, cnt = M_CHUNKS[r]
        if r == 0:
            s["ov"] = ps_out.tile([C, 512], f32, tag="out", name="ps_ov")
            s["z"] = ps_rs.tile([C, 512], f32, tag="rs", name="ps_z")
        ps_ov, ps_z = s["ov"], s["z"]
        pp = pp_of[b]
        for h in range(HEADS):
            nc.tensor.matmul(
                ps_ov[HD * h:HD * (h + 1), 0:NSL],
                lhsT=s["v"][0:cnt, r, h, :],
                rhs=pp[0:cnt, r, h, :],
                start=(r == 0), stop=(r == len(M_CHUNKS) - 1),
                tile_position=(0, HD * h), skip_group_check=True)
        for h in range(HEADS):
            nc.tensor.matmul(
                ps_z[HD * h:HD * (h + 1), 0:NSL],
                lhsT=ones_sb[0:cnt, :],
                rhs=pp[0:cnt, r, h, :],
                start=(r == 0), stop=(r == len(M_CHUNKS) - 1),
                tile_position=(0, HD * h), skip_group_check=True)

    def extract(b):
        """Normalize straight out of PSUM: recip(rowsums), multiply."""
        s = state[b]
        ps_ov = s.pop("ov")
        ps_z = s.pop("z")
        rb_sb = opool.tile([C, NSL], f32, tag="rb")
        nc.vector.reciprocal_approx_fast(rb_sb[:], ps_z[0:C, 0:NSL])
        outT_sb = opool.tile([C, NSL], pdt, tag="outT")
        s["outT"] = outT_sb
        nc.vector.tensor_mul(outT_sb[:], ps_ov[0:C, 0:NSL], rb_sb[:])

    def proj_tail(b):
        """Final projection in transposed layout; host untransposes."""
        s = state[b]
        ps_ft = ps_rot.tile([C, 2, 512], f32, tag="rot", name="ps_ft")
        ps_ft = ps_ft[:, 0, :]
        nc.tensor.matmul(ps_ft[0:C, 0:NSL], lhsT=wp_sb[:],
                         rhs=s.pop("outT")[:], start=True, stop=True)
        fin_sb = opool.tile([C, NSL], f32, tag="fin")
        nc.vector.tensor_scalar_add(fin_sb[:], ps_ft[0:C, 0:NSL],
                                    bp_sb[:, 0:1])
        nc.sync.dma_start(out=out_d[b], in_=fin_sb[:])
        state.pop(b)

    # ---- software pipeline ----
    prep_load(0)
    prep_q(0)
    for w in range(2):
        for mc in range(2):
            prep_tap(0, w, mc)
    prep_valloc(0)
    for j in range(7):
        prep_vtrans(0, j)
    prep_load(1)
    for b in range(B):
        pp_of[b] = ppool.tile([C, 7, HEADS, NSL], pdt, tag="pp", name="pp_sb")
        if b + 2 < B:
            prep_load(b + 2)
        for step in range(14):
            half_round(b, step // 2, step % 2)
            if b >= 1:
                if step % 2 == 0 and step <= 12:
                    attnv4(b - 1, step // 2)
                elif step == 13:
                    extract(b - 1)
            if b >= 2 and step == 0:
                proj_tail(b - 2)
            if b + 1 < B:
                if step == 2:
                    prep_q(b + 1)
                elif 3 <= step <= 6:
                    prep_tap(b + 1, (step - 3) // 2, (step - 3) % 2)
                elif step == 7:
                    prep_valloc(b + 1)
                elif 8 <= step <= 13:
                    prep_vtrans(b + 1, step - 8)
        if b + 1 < B:
            prep_vtrans(b + 1, 6)
        pp_of.pop(b - 2, None)
    # drain
    proj_tail(B - 2)
    for r in range(7):
        attnv4(B - 1, r)
    extract(B - 1)
    proj_tail(B - 1)


def _get_compiled():
    global _COMPILED
    if _COMPILED is None:
        _COMPILED = _build()
    return _COMPILED


def make_in_map(prep, j):
    return {
        "xT": prep["xT"],
        "xTn": np.ascontiguousarray(prep["xT"][:, :, j * NSL:(j + 1) * NSL]),
        "expRT": prep["expRT"][j],
        "Wq": prep["Wq"], "bq": prep["bq"],
        "Wktap": prep["Wk_tap"], "Wvtap": prep["Wv_tap"],
        "Wp": prep["Wp"], "bp": prep["bp"],
    }


def kernel(x, relative_pos, Wq, bq, Wk, bk, Wv, bv, conv_w, conv_b,
           bn_gamma, bn_beta, bn_mean, bn_var, Wp, bp, H=56, W=56,
           _trace=False):
    from concourse.bass_utils import run_bass_kernel_spmd

    prep = _host_prep(x, relative_pos, Wq, bq, Wk, bk, Wv, bv, conv_w,
                      conv_b, bn_gamma, bn_beta, bn_mean, bn_var, Wp, bp)
    nc = _get_compiled()

    in_maps = [make_in_map(prep, j) for j in range(NCORES)]

    res = run_bass_kernel_spmd(nc, in_maps, core_ids=list(range(NCORES)),
                               trace=_trace)

    out = np.empty((B, N, C), np.float32)
    for j in range(NCORES):
        out[:, j * NSL:(j + 1) * NSL, :] = \
            res.results[j]["out"].transpose(0, 2, 1)
    if _trace:
        kernel._last_result = res
    return out


# revision 30
# speedup vs baseline: 1.1584x; 1.1584x over previous
"""Trainium2 Bass kernel for PVT-style spatial-reduction attention.

Model (see reference):
  q = (x @ Wq + bq) * hd^-0.5                       (B, N, C) -> heads of 32
  x_ = BN(DWConv2x2s2(x)) ; k = x_ @ Wk + bk ; v = x_ @ Wv + bv
  attn = softmax(q k^T + rel_pos) ; out = (attn @ v) @ Wp + bp

Shapes: B=8, N=3136 (56x56), C=128, heads=4, hd=32, Nkv=784 (28x28).

Distribution: data-parallel over batch -- core j handles batch j fully
(B == n_cores == 8).  k/v/conv-taps are computed once per core (vs 8x
redundantly under query-sharding), cutting TensorE work by a third; the
exp(rel_pos) table is streamed per n-chunk from HBM instead.

Device layout strategy: features-on-partitions everywhere (C == 128).
  - host passes xT (B, C, N) in bf16; all projections are lhsT=weight
    matmuls.
  - conv+BN+k/v projection fused into 4 "tap" weight matrices (host
    precomputed); k-bias dropped (softmax-invariant), v-bias folded into
    the final bias.
  - scores computed transposed: S^T[m, n] per (nch, h); softmax uses
    exp(S + R) = exp(S) * exp(R) with exp(rel_pos^T) precomputed on host,
    interleaved per n-chunk, and double-buffer streamed.
  - per (n-chunk, kv-chunk): attn@v runs all 4 heads 4-way column-packed
    (32-wide) into ONE psum bank; row sums via 4 column-packed all-ones
    matmuls (32-replicated) into a second bank, which doubles as the
    softmax-denominator broadcast: extract = reciprocal + multiply only.
  - engine balance: ScalarE does exp only; tap PSUM->SBUF copies run on
    VectorE; 5 of 14 exp(R) multiplies per slot run on GpSimd.
  - software pipeline per n-chunk slot (14 half-round steps):
      all steps:   scores+exp+mul of slot (kv chunk step//2, pair step%2)
      even steps:  attn@v + rowsums of slot-1, one kv chunk per step
      step 13:     extract (normalize) of slot-1
      step 0:      projection tail of slot-2
      step 1:      q projection of slot+1; expTI prefetch of slot+2
  - final output is produced transposed (C, N) per core; the host
    untransposes while assembling the full (B, N, C) result.
"""

import os
import sys

import numpy as np

if "/opt/trn_rl_repo" not in sys.path:
    sys.path.insert(0, "/opt/trn_rl_repo")

B = 8
N = 3136
C = 128
HEADS = 4
HD = 32
SR = 2
H = W = 56
NKV = 784  # 28*28
NCORES = 8
NSL = N // NCORES  # 392 query rows per n-chunk slot
NCH = N // NSL     # 8 slots per core
BN_EPS = 1e-5
SCALE = HD ** -0.5

# m (kv index) chunking: 784 = 6*128 + 16
M_CHUNKS = [(j * 128, min(128, NKV - j * 128)) for j in range((NKV + 127) // 128)]

PROB_BF16 = os.environ.get("KERNEL_PROB_BF16", "1") == "1"
# (r, hp) half-rounds whose exp(R) multiply runs on GpSimd instead of DVE
GPS_MULS = {(r, 1) for r in range(5)}

_COMPILED = None  # cached nc across kernel() calls
_PREP_CACHE = {}  # host-prep results cached by input id


def _host_prep(x, relative_pos, Wq, bq, Wk, bk, Wv, bv, conv_w, conv_b,
               bn_gamma, bn_beta, bn_mean, bn_var, Wp, bp):
    """Fuse conv/BN into tap weights; fold biases; transpose activations."""
    import ml_dtypes
    f32 = np.float32
    bf16 = ml_dtypes.bfloat16
    wdt = bf16 if PROB_BF16 else f32
    x = np.asarray(x, f32)
    # xT: (B, C, N)
    xT = np.ascontiguousarray(x.transpose(0, 2, 1).astype(wdt))

    inv = (np.asarray(bn_gamma, f32)
           / np.sqrt(np.asarray(bn_var, f32) + BN_EPS))          # [c]
    wp_taps = np.asarray(conv_w, f32).reshape(C, SR * SR) * inv[:, None]  # [c,4]
    beta0 = (np.asarray(conv_b, f32) * inv
             + np.asarray(bn_beta, f32)
             - np.asarray(bn_mean, f32) * inv)                    # [c]

    Wk = np.asarray(Wk, f32)
    Wv = np.asarray(Wv, f32)
    # Wk_tap[t, c, c'] = wp_taps[c, t] * Wk[c, c']
    Wk_tap = np.ascontiguousarray(
        (wp_taps.T[:, :, None] * Wk[None, :, :]).astype(wdt))     # (4, C, C)
    Wv_tap = np.ascontiguousarray(
        (wp_taps.T[:, :, None] * Wv[None, :, :]).astype(wdt))

    # v bias (uniform over kv positions -> exact fold into final bias)
    beta_v = beta0 @ Wv + np.asarray(bv, f32)                     # [c']
    bp_col = (np.asarray(bp, f32) + beta_v @ np.asarray(Wp, f32)).reshape(C, 1)

    Wq_s = np.ascontiguousarray((np.asarray(Wq, f32) * SCALE).astype(wdt))
    bq_col = (np.asarray(bq, f32) * SCALE).reshape(C, 1)

    # exp(rel)^T interleaved per n-chunk: (NCH, C, 7, HEADS, NSL),
    # exactly the on-device expTI layout so each slot is one linear DMA.
    rel = np.asarray(relative_pos, f32)                  # (4, N, NKV)
    e = np.exp(rel).transpose(0, 2, 1).astype(wdt)       # (4, NKV, N)
    expI = np.zeros((NCH, C, 7, HEADS, NSL), wdt)
    for j, (m0, cnt) in enumerate(M_CHUNKS):
        # (4, cnt, NCH, NSL) -> (NCH, cnt, h, NSL)
        blk = e[:, m0:m0 + cnt, :].reshape(HEADS, cnt, NCH, NSL)
        expI[:, 0:cnt, j, :, :] = blk.transpose(2, 1, 0, 3)
    expI = np.ascontiguousarray(expI)

    return dict(xT=xT, Wk_tap=Wk_tap, Wv_tap=Wv_tap, Wq=Wq_s, bq=bq_col,
                Wp=np.ascontiguousarray(np.asarray(Wp, f32).astype(wdt)),
                bp=bp_col, expI=expI)


def _build():
    """Build + compile the SPMD bass program (same NEFF for all 8 cores)."""
    import concourse.bass as bass
    import concourse.tile as tile
    from concourse import bacc, mybir
    from concourse.masks import make_identity

    f32 = mybir.dt.float32
    f32r = mybir.dt.float32r
    pdt = mybir.dt.bfloat16 if PROB_BF16 else f32

    nc = bacc.Bacc("TRN2", target_bir_lowering=False, debug=False,
                   num_devices=NCORES)

    # ---- DRAM I/O ----
    xT_d = nc.dram_tensor("xT", [C, N], pdt, kind="ExternalInput").ap()
    expI_d = nc.dram_tensor("expI", [NCH, C, 7 * HEADS * NSL], pdt,
                            kind="ExternalInput").ap()
    Wq_d = nc.dram_tensor("Wq", [C, C], pdt, kind="ExternalInput").ap()
    bq_d = nc.dram_tensor("bq", [C, 1], f32, kind="ExternalInput").ap()
    Wktap_d = nc.dram_tensor("Wktap", [SR * SR, C, C], pdt,
                             kind="ExternalInput").ap()
    Wvtap_d = nc.dram_tensor("Wvtap", [SR * SR, C, C], pdt,
                             kind="ExternalInput").ap()
    Wp_d = nc.dram_tensor("Wp", [C, C], pdt, kind="ExternalInput").ap()
    bp_d = nc.dram_tensor("bp", [C, 1], f32, kind="ExternalInput").ap()
    out_d = nc.dram_tensor("out", [C, N], f32, kind="ExternalOutput").ap()

    with tile.TileContext(nc) as tc:
        from contextlib import ExitStack
        with ExitStack() as ctx:
            _emit(ctx, tc, nc, bass, mybir, make_identity, f32, f32r, pdt,
                  xT_d, expI_d, Wq_d, bq_d, Wktap_d, Wvtap_d,
                  Wp_d, bp_d, out_d)

    nc.compile()
    return nc


def _emit(ctx, tc, nc, bass, mybir, make_identity, f32, f32r, pdt,
          xT_d, expI_d, Wq_d, bq_d, Wktap_d, Wvtap_d, Wp_d, bp_d, out_d):
    AF = mybir.ActivationFunctionType

    singles = ctx.enter_context(tc.tile_pool(name="singles", bufs=1))
    ppool = ctx.enter_context(tc.tile_pool(name="ppool", bufs=3))
    epool = ctx.enter_context(tc.tile_pool(name="epool", bufs=3))
    opool = ctx.enter_context(tc.tile_pool(name="opool", bufs=3))
    qpool = ctx.enter_context(tc.tile_pool(name="qpool", bufs=3))
    ptpool = ctx.enter_context(tc.tile_pool(name="ptpool", bufs=6))
    # PSUM: rot 3x2 + out 1 + rs 1 = 8 banks
    ps_rot = ctx.enter_context(tc.tile_pool(name="ps_rot", bufs=3,
                                            space="PSUM"))
    ps_out = ctx.enter_context(tc.tile_pool(name="ps_out", bufs=1,
                                            space="PSUM"))
    ps_rs = ctx.enter_context(tc.tile_pool(name="ps_rs", bufs=1,
                                           space="PSUM"))

    # ---- constants ----
    identb = singles.tile([C, C], pdt)
    make_identity(nc, identb[:])
    ones_sb = singles.tile([C, HD], pdt)
    nc.vector.memset(ones_sb[:], 1.0)

    wq_sb = singles.tile([C, C], pdt)
    nc.sync.dma_start(out=wq_sb[:], in_=Wq_d)
    bq_sb = singles.tile([C, 1], f32)
    nc.sync.dma_start(out=bq_sb[:], in_=bq_d)
    wk_sb = singles.tile([C, SR * SR, C], pdt)
    nc.sync.dma_start(out=wk_sb[:], in_=Wktap_d.rearrange("t c d -> c t d"))
    wv_sb = singles.tile([C, SR * SR, C], pdt)
    nc.sync.dma_start(out=wv_sb[:], in_=Wvtap_d.rearrange("t c d -> c t d"))
    wp_sb = singles.tile([C, C], pdt)
    nc.sync.dma_start(out=wp_sb[:], in_=Wp_d)
    bp_sb = singles.tile([C, 1], f32)
    nc.sync.dma_start(out=bp_sb[:], in_=bp_d)

    # whole-batch activations + k/v, resident all kernel
    xT_sb = singles.tile([C, N], pdt)
    nc.sync.dma_start(out=xT_sb[:, 0:N // 2], in_=xT_d[:, 0:N // 2])
    nc.sync.dma_start(out=xT_sb[:, N // 2:N], in_=xT_d[:, N // 2:N])
    kT_sb = singles.tile([C, 7 * 128], pdt)
    nc.vector.memset(kT_sb[:, NKV:7 * 128], 0.0)
    vT_sb = singles.tile([C, NKV], pdt)
    v_sb = singles.tile([C, 7, HEADS, HD], pdt)

    state = {}
    pp_of = {}
    exp_of = {}

    def prep_tap(which, mc):
        """One kv-chunk of the fused conv-tap projection (4 matmuls)."""
        dst = kT_sb if which == 0 else vT_sb
        w_sb = wk_sb if which == 0 else wv_sb
        xview = xT_sb[:].rearrange("p (i a j c) -> p a c i j",
                                   i=28, a=2, j=28, c=2)
        ps_kv = ps_rot.tile([C, 2, 512], f32, tag="rot", name="ps_kv")
        ps_kv = ps_kv[:, 0, :]
        for t in range(SR * SR):
            di, dj = t // 2, t % 2
            rhs = xview[:, di, dj, 14 * mc:14 * mc + 14, :]
            nc.tensor.matmul(ps_kv[:, 0:392], lhsT=w_sb[:, t, :],
                             rhs=rhs, start=(t == 0), stop=(t == 3))
        nc.vector.tensor_copy(dst[:, 392 * mc:392 * (mc + 1)], ps_kv[:, 0:392])

    def prep_vtrans(j):
        m0, cnt = M_CHUNKS[j]
        ps_t = ps_rot.tile([C, 2, 512], pdt, tag="rot", name="ps_t")
        ps_t = ps_t[:, 0, :]
        nc.tensor.transpose(ps_t[0:cnt, 0:C], vT_sb[:, m0:m0 + cnt],
                            identb[:])
        nc.vector.tensor_copy(
            v_sb[0:cnt, j, :, :],
            ps_t[0:cnt, 0:C].rearrange("p (h d) -> p h d", h=HEADS, d=HD))

    def exp_load(nch):
        """Prefetch the exp(rel) interleave for slot nch (4 parallel DMAs)."""
        e_sb = epool.tile([C, 7, HEADS, NSL], pdt, tag="expTI", name="e_sb")
        exp_of[nch] = e_sb
        flat = e_sb[:].rearrange("p a h n -> p (a h n)")
        tot = 7 * HEADS * NSL
        qtr = tot // 4
        for i in range(4):
            nc.sync.dma_start(out=flat[:, i * qtr:(i + 1) * qtr],
                              in_=expI_d[nch, :, i * qtr:(i + 1) * qtr])

    def prep_q(nch):
        s = state.setdefault(nch, {})
        ps_q = ps_rot.tile([C, 2, 512], f32, tag="rot", name="ps_q")
        ps_q = ps_q[:, 0, :]
        nc.tensor.matmul(ps_q[:, 0:NSL], lhsT=wq_sb[:],
                         rhs=xT_sb[:, nch * NSL:(nch + 1) * NSL],
                         start=True, stop=True)
        qT_sb = qpool.tile([C, NSL], pdt, tag="qT", name="qT_sb")
        s["qT"] = qT_sb
        nc.vector.tensor_scalar_add(qT_sb[:], ps_q[:, 0:NSL], bq_sb[:, 0:1])

    def half_round(nch, r, hp):
        """Scores + exp + expR multiply for chunk r, head pair hp."""
        s = state[nch]
        ps_s = ps_rot.tile([C, 2, 512], f32, tag="rot", name="ps_s")
        for hh in range(2):
            h = 2 * hp + hh
            nc.tensor.matmul(
                ps_s[0:128, hh, 0:NSL],
                lhsT=kT_sb[HD * h:HD * (h + 1), 128 * r:128 * (r + 1)],
                rhs=s["qT"][HD * h:HD * (h + 1), :],
                start=True, stop=True,
                tile_position=(HD * h, 0))
        pt_sb = ptpool.tile([C, 2, NSL], pdt, tag="pt")
        nc.scalar.activation(pt_sb[:], ps_s[:, :, 0:NSL], AF.Exp)
        eng = nc.gpsimd if (r, hp) in GPS_MULS else nc.vector
        eng.tensor_mul(pp_of[nch][:, r, 2 * hp:2 * hp + 2, :], pt_sb[:],
                       exp_of[nch][:, r, 2 * hp:2 * hp + 2, :])

    def attnv4(nch, r):
        """attn@v + rowsums for kv chunk r: all 4 heads column-packed."""
        s = state[nch]
        m0, cnt = M_CHUNKS[r]
        if r == 0:
            s["ov"] = ps_out.tile([C, 512], f32, tag="out", name="ps_ov")
            s["z"] = ps_rs.tile([C, 512], f32, tag="rs", name="ps_z")
        ps_ov, ps_z = s["ov"], s["z"]
        pp = pp_of[nch]
        for h in range(HEADS):
            nc.tensor.matmul(
                ps_ov[HD * h:HD * (h + 1), 0:NSL],
                lhsT=v_sb[0:cnt, r, h, :],
                rhs=pp[0:cnt, r, h, :],
                start=(r == 0), stop=(r == len(M_CHUNKS) - 1),
                tile_position=(0, HD * h), skip_group_check=True)
        for h in range(HEADS):
            nc.tensor.matmul(
                ps_z[HD * h:HD * (h + 1), 0:NSL],
                lhsT=ones_sb[0:cnt, :],
                rhs=pp[0:cnt, r, h, :],
                start=(r == 0), stop=(r == len(M_CHUNKS) - 1),
                tile_position=(0, HD * h), skip_group_check=True)

    def extract(nch):
        """Normalize straight out of PSUM: recip(rowsums), multiply."""
        s = state[nch]
        ps_ov = s.pop("ov")
        ps_z = s.pop("z")
        rb_sb = opool.tile([C, NSL], f32, tag="rb")
        nc.vector.reciprocal_approx_fast(rb_sb[:], ps_z[0:C, 0:NSL])
        outT_sb = opool.tile([C, NSL], pdt, tag="outT")
        s["outT"] = outT_sb
        nc.vector.tensor_mul(outT_sb[:], ps_ov[0:C, 0:NSL], rb_sb[:])

    def proj_tail(nch):
        """Final projection in transposed layout; host untransposes."""
        s = state[nch]
        ps_ft = ps_rot.tile([C, 2, 512], f32, tag="rot", name="ps_ft")
        ps_ft = ps_ft[:, 0, :]
        nc.tensor.matmul(ps_ft[0:C, 0:NSL], lhsT=wp_sb[:],
                         rhs=s.pop("outT")[:], start=True, stop=True)
        fin_sb = opool.tile([C, NSL], f32, tag="fin")
        nc.vector.tensor_scalar_add(fin_sb[:], ps_ft[0:C, 0:NSL],
                                    bp_sb[:, 0:1])
        nc.sync.dma_start(out=out_d[:, nch * NSL:(nch + 1) * NSL],
                          in_=fin_sb[:])
        state.pop(nch)
        pp_of.pop(nch, None)
        exp_of.pop(nch, None)

    # ---- fill: k/v once, first exp tables, first q ----
    exp_load(0)
    for w in range(2):
        for mc in range(2):
            prep_tap(w, mc)
    exp_load(1)
    for j in range(7):
        prep_vtrans(j)
    prep_q(0)
    # ---- steady loop over n-chunk slots ----
    for nch in range(NCH):
        pp_of[nch] = ppool.tile([C, 7, HEADS, NSL], pdt, tag="pp",
                                name="pp_sb")
        for step in range(14):
            half_round(nch, step // 2, step % 2)
            if nch >= 1:
                if step % 2 == 0 and step <= 12:
                    attnv4(nch - 1, step // 2)
                elif step == 13:
                    extract(nch - 1)
            if nch >= 2 and step == 0:
                proj_tail(nch - 2)
            if step == 1:
                if nch + 1 < NCH:
                    prep_q(nch + 1)
                if nch + 2 < NCH:
                    exp_load(nch + 2)
    # drain
    proj_tail(NCH - 2)
    for r in range(7):
        attnv4(NCH - 1, r)
    extract(NCH - 1)
    proj_tail(NCH - 1)


def _get_compiled():
    global _COMPILED
    if _COMPILED is None:
        _COMPILED = _build()
    return _COMPILED


def make_in_map(prep, j):
    return {
        "xT": np.ascontiguousarray(prep["xT"][j]),
        "expI": prep["expI"].reshape(NCH, C, 7 * HEADS * NSL),
        "Wq": prep["Wq"], "bq": prep["bq"],
        "Wktap": prep["Wk_tap"], "Wvtap": prep["Wv_tap"],
        "Wp": prep["Wp"], "bp": prep["bp"],
    }


def kernel(x, relative_pos, Wq, bq, Wk, bk, Wv, bv, conv_w, conv_b,
           bn_gamma, bn_beta, bn_mean, bn_var, Wp, bp, H=56, W=56,
           _trace=False):
    from concourse.bass_utils import run_bass_kernel_spmd

    prep = _host_prep(x, relative_pos, Wq, bq, Wk, bk, Wv, bv, conv_w,
                      conv_b, bn_gamma, bn_beta, bn_mean, bn_var, Wp, bp)
    nc = _get_compiled()

    in_maps = [make_in_map(prep, j) for j in range(NCORES)]

    res = run_bass_kernel_spmd(nc, in_maps, core_ids=list(range(NCORES)),
                               trace=_trace)

    out = np.empty((B, N, C), np.float32)
    for j in range(NCORES):
        out[j] = res.results[j]["out"].T
    if _trace:
        kernel._last_result = res
    return out


# revision 33
# speedup vs baseline: 1.1963x; 1.0327x over previous
"""Trainium2 Bass kernel for PVT-style spatial-reduction attention.

Model (see reference):
  q = (x @ Wq + bq) * hd^-0.5                       (B, N, C) -> heads of 32
  x_ = BN(DWConv2x2s2(x)) ; k = x_ @ Wk + bk ; v = x_ @ Wv + bv
  attn = softmax(q k^T + rel_pos) ; out = (attn @ v) @ Wp + bp

Shapes: B=8, N=3136 (56x56), C=128, heads=4, hd=32, Nkv=784 (28x28).

Distribution: data-parallel over batch -- core j handles batch j fully
(B == n_cores == 8).  k/v/conv-taps are computed once per core (vs 8x
redundantly under query-sharding), cutting TensorE work by a third; the
exp(rel_pos) table is streamed per n-chunk from HBM instead.

Device layout strategy: features-on-partitions everywhere (C == 128).
  - host passes xT (B, C, N) in bf16; all projections are lhsT=weight
    matmuls.
  - conv+BN+k/v projection fused into 4 "tap" weight matrices (host
    precomputed); k-bias dropped (softmax-invariant), v-bias folded into
    the final bias.
  - scores computed transposed: S^T[m, n] per (nch, h); softmax uses
    exp(S + R) = exp(S) * exp(R) with exp(rel_pos^T) precomputed on host,
    interleaved per n-chunk, and double-buffer streamed.
  - per (n-chunk, kv-chunk): attn@v runs all 4 heads 4-way column-packed
    (32-wide) into ONE psum bank; row sums via 4 column-packed all-ones
    matmuls (32-replicated) into a second bank, which doubles as the
    softmax-denominator broadcast: extract = reciprocal + multiply only.
  - engine balance: ScalarE does exp only; tap PSUM->SBUF copies run on
    VectorE; 5 of 14 exp(R) multiplies per slot run on GpSimd.
  - software pipeline per n-chunk slot (14 half-round steps):
      all steps:   scores+exp+mul of slot (kv chunk step//2, pair step%2)
      even steps:  attn@v + rowsums of slot-1, one kv chunk per step
      step 13:     extract (normalize) of slot-1
      step 0:      projection tail of slot-2
      step 1:      q projection of slot+1; expTI prefetch of slot+2
  - final output is produced transposed (C, N) per core; the host
    untransposes while assembling the full (B, N, C) result.
"""

import os
import sys

import numpy as np

if "/opt/trn_rl_repo" not in sys.path:
    sys.path.insert(0, "/opt/trn_rl_repo")

B = 8
N = 3136
C = 128
HEADS = 4
HD = 32
SR = 2
H = W = 56
NKV = 784  # 28*28
NCORES = 8
NSL = N // NCORES  # 392 query rows per n-chunk slot
NCH = N // NSL     # 8 slots per core
BN_EPS = 1e-5
SCALE = HD ** -0.5

# m (kv index) chunking: 784 = 6*128 + 16
M_CHUNKS = [(j * 128, min(128, NKV - j * 128)) for j in range((NKV + 127) // 128)]

PROB_BF16 = os.environ.get("KERNEL_PROB_BF16", "1") == "1"
# (r, hp) half-rounds whose exp(R) multiply runs on GpSimd instead of DVE
GPS_MULS = {(r, 1) for r in range(5)}

_COMPILED = None  # cached nc across kernel() calls
_PREP_CACHE = {}  # host-prep results cached by input id


def _host_prep(x, relative_pos, Wq, bq, Wk, bk, Wv, bv, conv_w, conv_b,
               bn_gamma, bn_beta, bn_mean, bn_var, Wp, bp):
    """Fuse conv/BN into tap weights; fold biases; transpose activations."""
    import ml_dtypes
    f32 = np.float32
    bf16 = ml_dtypes.bfloat16
    wdt = bf16 if PROB_BF16 else f32
    x = np.asarray(x, f32)
    # xT: (B, C, N)
    xT = np.ascontiguousarray(x.transpose(0, 2, 1).astype(wdt))

    inv = (np.asarray(bn_gamma, f32)
           / np.sqrt(np.asarray(bn_var, f32) + BN_EPS))          # [c]
    wp_taps = np.asarray(conv_w, f32).reshape(C, SR * SR) * inv[:, None]  # [c,4]
    beta0 = (np.asarray(conv_b, f32) * inv
             + np.asarray(bn_beta, f32)
             - np.asarray(bn_mean, f32) * inv)                    # [c]

    Wk = np.asarray(Wk, f32)
    Wv = np.asarray(Wv, f32)
    # Wk_tap[t, c, c'] = wp_taps[c, t] * Wk[c, c']
    Wk_tap = np.ascontiguousarray(
        (wp_taps.T[:, :, None] * Wk[None, :, :]).astype(wdt))     # (4, C, C)
    Wv_tap = np.ascontiguousarray(
        (wp_taps.T[:, :, None] * Wv[None, :, :]).astype(wdt))

    # v bias (uniform over kv positions -> exact fold into final bias)
    beta_v = beta0 @ Wv + np.asarray(bv, f32)                     # [c']
    bp_col = (np.asarray(bp, f32) + beta_v @ np.asarray(Wp, f32)).reshape(C, 1)

    Wq_s = np.ascontiguousarray((np.asarray(Wq, f32) * SCALE).astype(wdt))
    bq_col = (np.asarray(bq, f32) * SCALE).reshape(C, 1)

    # exp(rel)^T interleaved per n-chunk: (NCH, C, 7, HEADS, NSL),
    # exactly the on-device expTI layout so each slot is one linear DMA.
    rel = np.asarray(relative_pos, f32)                  # (4, N, NKV)
    e = np.exp(rel).transpose(0, 2, 1).astype(wdt)       # (4, NKV, N)
    expI = np.zeros((NCH, C, 7, HEADS, NSL), wdt)
    for j, (m0, cnt) in enumerate(M_CHUNKS):
        # (4, cnt, NCH, NSL) -> (NCH, cnt, h, NSL)
        blk = e[:, m0:m0 + cnt, :].reshape(HEADS, cnt, NCH, NSL)
        expI[:, 0:cnt, j, :, :] = blk.transpose(2, 1, 0, 3)
    expI = np.ascontiguousarray(expI)

    return dict(xT=xT, Wk_tap=Wk_tap, Wv_tap=Wv_tap, Wq=Wq_s, bq=bq_col,
                Wp=np.ascontiguousarray(np.asarray(Wp, f32).astype(wdt)),
                bp=bp_col, expI=expI)


def _build():
    """Build + compile the SPMD bass program (same NEFF for all 8 cores)."""
    import concourse.bass as bass
    import concourse.tile as tile
    from concourse import bacc, mybir
    from concourse.masks import make_identity

    f32 = mybir.dt.float32
    f32r = mybir.dt.float32r
    pdt = mybir.dt.bfloat16 if PROB_BF16 else f32

    nc = bacc.Bacc("TRN2", target_bir_lowering=False, debug=False,
                   num_devices=NCORES)

    # ---- DRAM I/O ----
    xT_d = nc.dram_tensor("xT", [C, N], pdt, kind="ExternalInput").ap()
    expI_d = nc.dram_tensor("expI", [NCH, C, 7 * HEADS * NSL], pdt,
                            kind="ExternalInput").ap()
    Wq_d = nc.dram_tensor("Wq", [C, C], pdt, kind="ExternalInput").ap()
    bq_d = nc.dram_tensor("bq", [C, 1], f32, kind="ExternalInput").ap()
    Wktap_d = nc.dram_tensor("Wktap", [SR * SR, C, C], pdt,
                             kind="ExternalInput").ap()
    Wvtap_d = nc.dram_tensor("Wvtap", [SR * SR, C, C], pdt,
                             kind="ExternalInput").ap()
    Wp_d = nc.dram_tensor("Wp", [C, C], pdt, kind="ExternalInput").ap()
    bp_d = nc.dram_tensor("bp", [C, 1], f32, kind="ExternalInput").ap()
    out_d = nc.dram_tensor("out", [C, N], f32, kind="ExternalOutput").ap()

    with tile.TileContext(nc) as tc:
        from contextlib import ExitStack
        with ExitStack() as ctx:
            _emit(ctx, tc, nc, bass, mybir, make_identity, f32, f32r, pdt,
                  xT_d, expI_d, Wq_d, bq_d, Wktap_d, Wvtap_d,
                  Wp_d, bp_d, out_d)

    nc.compile()
    return nc


def _emit(ctx, tc, nc, bass, mybir, make_identity, f32, f32r, pdt,
          xT_d, expI_d, Wq_d, bq_d, Wktap_d, Wvtap_d, Wp_d, bp_d, out_d):
    AF = mybir.ActivationFunctionType

    singles = ctx.enter_context(tc.tile_pool(name="singles", bufs=1))
    ppool = ctx.enter_context(tc.tile_pool(name="ppool", bufs=3))
    epool = ctx.enter_context(tc.tile_pool(name="epool", bufs=3))
    opool = ctx.enter_context(tc.tile_pool(name="opool", bufs=3))
    qpool = ctx.enter_context(tc.tile_pool(name="qpool", bufs=3))
    ptpool = ctx.enter_context(tc.tile_pool(name="ptpool", bufs=6))
    # PSUM: rot 3x2 + out 1 + rs 1 = 8 banks
    ps_rot = ctx.enter_context(tc.tile_pool(name="ps_rot", bufs=3,
                                            space="PSUM"))
    ps_out = ctx.enter_context(tc.tile_pool(name="ps_out", bufs=1,
                                            space="PSUM"))
    ps_rs = ctx.enter_context(tc.tile_pool(name="ps_rs", bufs=1,
                                           space="PSUM"))

    # ---- constants ----
    identb = singles.tile([C, C], pdt)
    make_identity(nc, identb[:])
    ones_sb = singles.tile([C, HD], pdt)
    nc.vector.memset(ones_sb[:], 1.0)

    wq_sb = singles.tile([C, C], pdt)
    nc.sync.dma_start(out=wq_sb[:], in_=Wq_d)
    bq_sb = singles.tile([C, 1], f32)
    nc.sync.dma_start(out=bq_sb[:], in_=bq_d)
    wk_sb = singles.tile([C, SR * SR, C], pdt)
    nc.sync.dma_start(out=wk_sb[:], in_=Wktap_d.rearrange("t c d -> c t d"))
    wv_sb = singles.tile([C, SR * SR, C], pdt)
    nc.sync.dma_start(out=wv_sb[:], in_=Wvtap_d.rearrange("t c d -> c t d"))
    wp_sb = singles.tile([C, C], pdt)
    nc.sync.dma_start(out=wp_sb[:], in_=Wp_d)
    bp_sb = singles.tile([C, 1], f32)
    nc.sync.dma_start(out=bp_sb[:], in_=bp_d)

    # whole-batch activations + k/v, resident all kernel
    xT_sb = singles.tile([C, N], pdt)
    nc.sync.dma_start(out=xT_sb[:, 0:N // 2], in_=xT_d[:, 0:N // 2])
    nc.sync.dma_start(out=xT_sb[:, N // 2:N], in_=xT_d[:, N // 2:N])
    kT_sb = singles.tile([C, 7 * 128], pdt)
    nc.vector.memset(kT_sb[:, NKV:7 * 128], 0.0)
    vT_sb = singles.tile([C, NKV], pdt)
    v_sb = singles.tile([C, 7, HEADS, HD], pdt)

    state = {}
    pp_of = {}
    exp_of = {}

    def prep_tap(which, mc):
        """One kv-chunk of the fused conv-tap projection (4 matmuls)."""
        dst = kT_sb if which == 0 else vT_sb
        w_sb = wk_sb if which == 0 else wv_sb
        xview = xT_sb[:].rearrange("p (i a j c) -> p a c i j",
                                   i=28, a=2, j=28, c=2)
        ps_kv = ps_rot.tile([C, 2, 512], f32, tag="rot", name="ps_kv")
        ps_kv = ps_kv[:, 0, :]
        for t in range(SR * SR):
            di, dj = t // 2, t % 2
            rhs = xview[:, di, dj, 14 * mc:14 * mc + 14, :]
            nc.tensor.matmul(ps_kv[:, 0:392], lhsT=w_sb[:, t, :],
                             rhs=rhs, start=(t == 0), stop=(t == 3))
        nc.vector.tensor_copy(dst[:, 392 * mc:392 * (mc + 1)], ps_kv[:, 0:392])

    def prep_vtrans(j):
        m0, cnt = M_CHUNKS[j]
        ps_t = ps_rot.tile([C, 2, 512], pdt, tag="rot", name="ps_t")
        ps_t = ps_t[:, 0, :]
        nc.tensor.transpose(ps_t[0:cnt, 0:C], vT_sb[:, m0:m0 + cnt],
                            identb[:])
        nc.vector.tensor_copy(
            v_sb[0:cnt, j, :, :],
            ps_t[0:cnt, 0:C].rearrange("p (h d) -> p h d", h=HEADS, d=HD))

    def exp_load(nch):
        """Prefetch the exp(rel) interleave for slot nch (4 parallel DMAs)."""
        e_sb = epool.tile([C, 7, HEADS, NSL], pdt, tag="expTI", name="e_sb")
        exp_of[nch] = e_sb
        flat = e_sb[:].rearrange("p a h n -> p (a h n)")
        tot = 7 * HEADS * NSL
        qtr = tot // 4
        for i in range(4):
            nc.sync.dma_start(out=flat[:, i * qtr:(i + 1) * qtr],
                              in_=expI_d[nch, :, i * qtr:(i + 1) * qtr])

    qT_sb = singles.tile([C, N], pdt)

    def prep_q(qc):
        """Whole-batch q projection, one 784-column piece at fill time."""
        ps_q = ps_rot.tile([C, 2, 512], f32, tag="rot", name="ps_q")
        for half in range(2):
            c0 = (2 * qc + half) * NSL
            nc.tensor.matmul(ps_q[:, half, 0:NSL], lhsT=wq_sb[:],
                             rhs=xT_sb[:, c0:c0 + NSL],
                             start=True, stop=True)
        nc.vector.tensor_scalar_add(qT_sb[:, 2 * qc * NSL:2 * (qc + 1) * NSL],
                                    ps_q[:, :, 0:NSL], bq_sb[:, 0:1])

    def scores_round(nch, r):
        """All 4 heads' score matmuls co-issued: 4-way row packing."""
        s = state.setdefault(nch, {})
        tiles = []
        for hp in range(2):
            ps_s = ps_rot.tile([C, 2, 512], f32, tag="rot", name="ps_s")
            tiles.append(ps_s)
        for h in range(HEADS):
            nc.tensor.matmul(
                tiles[h // 2][0:128, h % 2, 0:NSL],
                lhsT=kT_sb[HD * h:HD * (h + 1), 128 * r:128 * (r + 1)],
                rhs=qT_sb[HD * h:HD * (h + 1), nch * NSL:(nch + 1) * NSL],
                start=True, stop=True,
                tile_position=(HD * h, 0))
        s["sco"] = tiles

    def exp_mul(nch, r, hp):
        """exp + expR multiply for chunk r, head pair hp."""
        s = state[nch]
        ps_s = s["sco"][hp]
        pt_sb = ptpool.tile([C, 2, NSL], pdt, tag="pt")
        nc.scalar.activation(pt_sb[:], ps_s[:, :, 0:NSL], AF.Exp)
        eng = nc.gpsimd if (r, hp) in GPS_MULS else nc.vector
        eng.tensor_mul(pp_of[nch][:, r, 2 * hp:2 * hp + 2, :], pt_sb[:],
                       exp_of[nch][:, r, 2 * hp:2 * hp + 2, :])

    def attnv4(nch, r):
        """attn@v + rowsums for kv chunk r: all 4 heads column-packed."""
        s = state[nch]
        m0, cnt = M_CHUNKS[r]
        if r == 0:
            s["ov"] = ps_out.tile([C, 512], f32, tag="out", name="ps_ov")
            s["z"] = ps_rs.tile([C, 512], f32, tag="rs", name="ps_z")
        ps_ov, ps_z = s["ov"], s["z"]
        pp = pp_of[nch]
        for h in range(HEADS):
            nc.tensor.matmul(
                ps_ov[HD * h:HD * (h + 1), 0:NSL],
                lhsT=v_sb[0:cnt, r, h, :],
                rhs=pp[0:cnt, r, h, :],
                start=(r == 0), stop=(r == len(M_CHUNKS) - 1),
                tile_position=(0, HD * h), skip_group_check=True)
        for h in range(HEADS):
            nc.tensor.matmul(
                ps_z[HD * h:HD * (h + 1), 0:NSL],
                lhsT=ones_sb[0:cnt, :],
                rhs=pp[0:cnt, r, h, :],
                start=(r == 0), stop=(r == len(M_CHUNKS) - 1),
                tile_position=(0, HD * h), skip_group_check=True)

    def extract(nch):
        """Normalize straight out of PSUM: recip(rowsums), multiply."""
        s = state[nch]
        ps_ov = s.pop("ov")
        ps_z = s.pop("z")
        rb_sb = opool.tile([C, NSL], f32, tag="rb")
        nc.vector.reciprocal_approx_fast(rb_sb[:], ps_z[0:C, 0:NSL])
        outT_sb = opool.tile([C, NSL], pdt, tag="outT")
        s["outT"] = outT_sb
        nc.vector.tensor_mul(outT_sb[:], ps_ov[0:C, 0:NSL], rb_sb[:])

    def proj_tail(nch):
        """Final projection in transposed layout; host untransposes."""
        s = state[nch]
        ps_ft = ps_rot.tile([C, 2, 512], f32, tag="rot", name="ps_ft")
        ps_ft = ps_ft[:, 0, :]
        nc.tensor.matmul(ps_ft[0:C, 0:NSL], lhsT=wp_sb[:],
                         rhs=s.pop("outT")[:], start=True, stop=True)
        fin_sb = opool.tile([C, NSL], f32, tag="fin")
        nc.vector.tensor_scalar_add(fin_sb[:], ps_ft[0:C, 0:NSL],
                                    bp_sb[:, 0:1])
        nc.sync.dma_start(out=out_d[:, nch * NSL:(nch + 1) * NSL],
                          in_=fin_sb[:])
        state.pop(nch)
        pp_of.pop(nch, None)
        exp_of.pop(nch, None)

    # ---- fill: k/v + all of q once, first exp tables ----
    exp_load(0)
    for w in range(2):
        for mc in range(2):
            prep_tap(w, mc)
    exp_load(1)
    for qc in range(4):
        prep_q(qc)
    for j in range(7):
        prep_vtrans(j)
    # ---- steady loop over n-chunk slots ----
    for nch in range(NCH):
        pp_of[nch] = ppool.tile([C, 7, HEADS, NSL], pdt, tag="pp",
                                name="pp_sb")
        for step in range(14):
            r, hp = step // 2, step % 2
            if hp == 0:
                scores_round(nch, r)
            exp_mul(nch, r, hp)
            if nch >= 1:
                if step % 2 == 0 and step <= 12:
                    attnv4(nch - 1, step // 2)
                elif step == 13:
                    extract(nch - 1)
            if nch >= 2 and step == 0:
                proj_tail(nch - 2)
            if step == 1 and nch + 2 < NCH:
                exp_load(nch + 2)
    # drain
    proj_tail(NCH - 2)
    for r in range(7):
        attnv4(NCH - 1, r)
    extract(NCH - 1)
    proj_tail(NCH - 1)


def _get_compiled():
    global _COMPILED
    if _COMPILED is None:
        _COMPILED = _build()
    return _COMPILED


def make_in_map(prep, j):
    return {
        "xT": np.ascontiguousarray(prep["xT"][j]),
        "expI": prep["expI"].reshape(NCH, C, 7 * HEADS * NSL),
        "Wq": prep["Wq"], "bq": prep["bq"],
        "Wktap": prep["Wk_tap"], "Wvtap": prep["Wv_tap"],
        "Wp": prep["Wp"], "bp": prep["bp"],
    }


def kernel(x, relative_pos, Wq, bq, Wk, bk, Wv, bv, conv_w, conv_b,
           bn_gamma, bn_beta, bn_mean, bn_var, Wp, bp, H=56, W=56,
           _trace=False):
    from concourse.bass_utils import run_bass_kernel_spmd

    prep = _host_prep(x, relative_pos, Wq, bq, Wk, bk, Wv, bv, conv_w,
                      conv_b, bn_gamma, bn_beta, bn_mean, bn_var, Wp, bp)
    nc = _get_compiled()

    in_maps = [make_in_map(prep, j) for j in range(NCORES)]

    res = run_bass_kernel_spmd(nc, in_maps, core_ids=list(range(NCORES)),
                               trace=_trace)

    out = np.empty((B, N, C), np.float32)
    for j in range(NCORES):
        out[j] = res.results[j]["out"].T
    if _trace:
        kernel._last_result = res
    return out


# revision 38
# speedup vs baseline: 1.2301x; 1.0283x over previous
"""Trainium2 Bass kernel for PVT-style spatial-reduction attention.

Model (see reference):
  q = (x @ Wq + bq) * hd^-0.5                       (B, N, C) -> heads of 32
  x_ = BN(DWConv2x2s2(x)) ; k = x_ @ Wk + bk ; v = x_ @ Wv + bv
  attn = softmax(q k^T + rel_pos) ; out = (attn @ v) @ Wp + bp

Shapes: B=8, N=3136 (56x56), C=128, heads=4, hd=32, Nkv=784 (28x28).

Distribution: data-parallel over batch -- core j handles batch j fully
(B == n_cores == 8).  k/v/conv-taps are computed once per core (vs 8x
redundantly under query-sharding), cutting TensorE work by a third; the
exp(rel_pos) table is streamed per n-chunk from HBM instead.

Device layout strategy: features-on-partitions everywhere (C == 128).
  - host passes xT (B, C, N) in bf16; all projections are lhsT=weight
    matmuls.
  - conv+BN+k/v projection fused into 4 "tap" weight matrices (host
    precomputed); k-bias dropped (softmax-invariant), v-bias folded into
    the final bias.
  - scores computed transposed: S^T[m, n] per (nch, h); softmax uses
    exp(S + R) = exp(S) * exp(R) with exp(rel_pos^T) precomputed on host,
    interleaved per n-chunk, and double-buffer streamed.
  - per (n-chunk, kv-chunk): attn@v runs all 4 heads 4-way column-packed
    (32-wide) into ONE psum bank; row sums via 4 column-packed all-ones
    matmuls (32-replicated) into a second bank, which doubles as the
    softmax-denominator broadcast: extract = reciprocal + multiply only.
  - engine balance: ScalarE does exp only; tap PSUM->SBUF copies run on
    VectorE; 5 of 14 exp(R) multiplies per slot run on GpSimd.
  - software pipeline per n-chunk slot (14 half-round steps):
      all steps:   scores+exp+mul of slot (kv chunk step//2, pair step%2)
      even steps:  attn@v + rowsums of slot-1, one kv chunk per step
      step 13:     extract (normalize) of slot-1
      step 0:      projection tail of slot-2
      step 1:      q projection of slot+1; expTI prefetch of slot+2
  - final output is produced transposed (C, N) per core; the host
    untransposes while assembling the full (B, N, C) result.
"""

import os
import sys

import numpy as np

if "/opt/trn_rl_repo" not in sys.path:
    sys.path.insert(0, "/opt/trn_rl_repo")

B = 8
N = 3136
C = 128
HEADS = 4
HD = 32
SR = 2
H = W = 56
NKV = 784  # 28*28
NCORES = 8
NSL = N // NCORES  # 392 query rows per n-chunk slot
NCH = N // NSL     # 8 slots per core
BN_EPS = 1e-5
SCALE = HD ** -0.5

# m (kv index) chunking: 784 = 6*128 + 16
M_CHUNKS = [(j * 128, min(128, NKV - j * 128)) for j in range((NKV + 127) // 128)]

PROB_BF16 = os.environ.get("KERNEL_PROB_BF16", "1") == "1"
# (r, hp) half-rounds whose exp(R) multiply runs on GpSimd instead of DVE
GPS_MULS = {(r, 1) for r in range(5)}

_COMPILED = None  # cached nc across kernel() calls
_PREP_CACHE = {}  # host-prep results cached by input id


def _host_prep(x, relative_pos, Wq, bq, Wk, bk, Wv, bv, conv_w, conv_b,
               bn_gamma, bn_beta, bn_mean, bn_var, Wp, bp):
    """Fuse conv/BN into tap weights; fold biases; transpose activations."""
    import ml_dtypes
    f32 = np.float32
    bf16 = ml_dtypes.bfloat16
    wdt = bf16 if PROB_BF16 else f32
    x = np.asarray(x, f32)
    # xT: (B, C, N)
    xT = np.ascontiguousarray(x.transpose(0, 2, 1).astype(wdt))

    inv = (np.asarray(bn_gamma, f32)
           / np.sqrt(np.asarray(bn_var, f32) + BN_EPS))          # [c]
    wp_taps = np.asarray(conv_w, f32).reshape(C, SR * SR) * inv[:, None]  # [c,4]
    beta0 = (np.asarray(conv_b, f32) * inv
             + np.asarray(bn_beta, f32)
             - np.asarray(bn_mean, f32) * inv)                    # [c]

    Wk = np.asarray(Wk, f32)
    Wv = np.asarray(Wv, f32)
    # Wk_tap[t, c, c'] = wp_taps[c, t] * Wk[c, c']
    Wk_tap = np.ascontiguousarray(
        (wp_taps.T[:, :, None] * Wk[None, :, :]).astype(wdt))     # (4, C, C)
    Wv_tap = np.ascontiguousarray(
        (wp_taps.T[:, :, None] * Wv[None, :, :]).astype(wdt))

    # v bias (uniform over kv positions -> exact fold into final bias)
    beta_v = beta0 @ Wv + np.asarray(bv, f32)                     # [c']
    bp_col = (np.asarray(bp, f32) + beta_v @ np.asarray(Wp, f32)).reshape(C, 1)

    Wq_s = np.ascontiguousarray((np.asarray(Wq, f32) * SCALE).astype(wdt))
    bq_col = (np.asarray(bq, f32) * SCALE).reshape(C, 1)

    # exp(rel)^T interleaved per n-chunk: (NCH, C, 7, HEADS, NSL),
    # exactly the on-device expTI layout so each slot is one linear DMA.
    rel = np.asarray(relative_pos, f32)                  # (4, N, NKV)
    e = np.exp(rel).transpose(0, 2, 1).astype(wdt)       # (4, NKV, N)
    expI = np.zeros((NCH, C, 7, HEADS, NSL), wdt)
    for j, (m0, cnt) in enumerate(M_CHUNKS):
        # (4, cnt, NCH, NSL) -> (NCH, cnt, h, NSL)
        blk = e[:, m0:m0 + cnt, :].reshape(HEADS, cnt, NCH, NSL)
        expI[:, 0:cnt, j, :, :] = blk.transpose(2, 1, 0, 3)
    expI = np.ascontiguousarray(expI)

    return dict(xT=xT, Wk_tap=Wk_tap, Wv_tap=Wv_tap, Wq=Wq_s, bq=bq_col,
                Wp=np.ascontiguousarray(np.asarray(Wp, f32).astype(wdt)),
                bp=bp_col, expI=expI)


def _build():
    """Build + compile the SPMD bass program (same NEFF for all 8 cores)."""
    import concourse.bass as bass
    import concourse.tile as tile
    from concourse import bacc, mybir
    from concourse.masks import make_identity

    f32 = mybir.dt.float32
    f32r = mybir.dt.float32r
    pdt = mybir.dt.bfloat16 if PROB_BF16 else f32

    nc = bacc.Bacc("TRN2", target_bir_lowering=False, debug=False,
                   num_devices=NCORES)

    # ---- DRAM I/O ----
    xT_d = nc.dram_tensor("xT", [C, N], pdt, kind="ExternalInput").ap()
    expI_d = nc.dram_tensor("expI", [NCH, C, 7 * HEADS * NSL], pdt,
                            kind="ExternalInput").ap()
    Wq_d = nc.dram_tensor("Wq", [C, C], pdt, kind="ExternalInput").ap()
    bq_d = nc.dram_tensor("bq", [C, 1], f32, kind="ExternalInput").ap()
    Wktap_d = nc.dram_tensor("Wktap", [SR * SR, C, C], pdt,
                             kind="ExternalInput").ap()
    Wvtap_d = nc.dram_tensor("Wvtap", [SR * SR, C, C], pdt,
                             kind="ExternalInput").ap()
    Wp_d = nc.dram_tensor("Wp", [C, C], pdt, kind="ExternalInput").ap()
    bp_d = nc.dram_tensor("bp", [C, 1], f32, kind="ExternalInput").ap()
    out_d = nc.dram_tensor("out", [C, N], f32, kind="ExternalOutput").ap()

    with tile.TileContext(nc) as tc:
        from contextlib import ExitStack
        with ExitStack() as ctx:
            _emit(ctx, tc, nc, bass, mybir, make_identity, f32, f32r, pdt,
                  xT_d, expI_d, Wq_d, bq_d, Wktap_d, Wvtap_d,
                  Wp_d, bp_d, out_d)

    nc.compile()
    return nc


def _emit(ctx, tc, nc, bass, mybir, make_identity, f32, f32r, pdt,
          xT_d, expI_d, Wq_d, bq_d, Wktap_d, Wvtap_d, Wp_d, bp_d, out_d):
    AF = mybir.ActivationFunctionType

    singles = ctx.enter_context(tc.tile_pool(name="singles", bufs=1))
    ppool = ctx.enter_context(tc.tile_pool(name="ppool", bufs=3))
    epool = ctx.enter_context(tc.tile_pool(name="epool", bufs=3))
    opool = ctx.enter_context(tc.tile_pool(name="opool", bufs=3))
    qpool = ctx.enter_context(tc.tile_pool(name="qpool", bufs=3))
    ptpool = ctx.enter_context(tc.tile_pool(name="ptpool", bufs=6))
    # PSUM: rot 3x2 + out 1 + rs 1 = 8 banks
    ps_rot = ctx.enter_context(tc.tile_pool(name="ps_rot", bufs=3,
                                            space="PSUM"))
    ps_out = ctx.enter_context(tc.tile_pool(name="ps_out", bufs=1,
                                            space="PSUM"))
    ps_rs = ctx.enter_context(tc.tile_pool(name="ps_rs", bufs=1,
                                           space="PSUM"))

    # ---- constants ----
    identb = singles.tile([C, C], pdt)
    make_identity(nc, identb[:])
    ones_sb = singles.tile([C, HD], pdt)
    nc.vector.memset(ones_sb[:], 1.0)

    wq_sb = singles.tile([C, C], pdt)
    nc.sync.dma_start(out=wq_sb[:], in_=Wq_d)
    bq_sb = singles.tile([C, 1], f32)
    nc.sync.dma_start(out=bq_sb[:], in_=bq_d)
    wk_sb = singles.tile([C, SR * SR, C], pdt)
    nc.sync.dma_start(out=wk_sb[:], in_=Wktap_d.rearrange("t c d -> c t d"))
    wv_sb = singles.tile([C, SR * SR, C], pdt)
    nc.sync.dma_start(out=wv_sb[:], in_=Wvtap_d.rearrange("t c d -> c t d"))
    wp_sb = singles.tile([C, C], pdt)
    nc.sync.dma_start(out=wp_sb[:], in_=Wp_d)
    bp_sb = singles.tile([C, 1], f32)
    nc.sync.dma_start(out=bp_sb[:], in_=bp_d)

    # whole-batch activations + k/v, resident all kernel
    xT_sb = singles.tile([C, N], pdt)
    nc.sync.dma_start(out=xT_sb[:, 0:N // 2], in_=xT_d[:, 0:N // 2])
    nc.sync.dma_start(out=xT_sb[:, N // 2:N], in_=xT_d[:, N // 2:N])
    kT_sb = singles.tile([C, 7 * 128], pdt)
    nc.vector.memset(kT_sb[:, NKV:7 * 128], 0.0)
    vT_sb = singles.tile([C, NKV], pdt)
    v_sb = singles.tile([C, 7, HEADS, HD], pdt)

    state = {}
    pp_of = {}
    exp_of = {}

    def prep_tap(which, mc):
        """One kv-chunk of the fused conv-tap projection (4 matmuls)."""
        dst = kT_sb if which == 0 else vT_sb
        w_sb = wk_sb if which == 0 else wv_sb
        xview = xT_sb[:].rearrange("p (i a j c) -> p a c i j",
                                   i=28, a=2, j=28, c=2)
        ps_kv = ps_rot.tile([C, 2, 512], f32, tag="rot", name="ps_kv")
        ps_kv = ps_kv[:, 0, :]
        for t in range(SR * SR):
            di, dj = t // 2, t % 2
            rhs = xview[:, di, dj, 14 * mc:14 * mc + 14, :]
            nc.tensor.matmul(ps_kv[:, 0:392], lhsT=w_sb[:, t, :],
                             rhs=rhs, start=(t == 0), stop=(t == 3))
        nc.vector.tensor_copy(dst[:, 392 * mc:392 * (mc + 1)], ps_kv[:, 0:392])

    def prep_vtrans(j):
        m0, cnt = M_CHUNKS[j]
        ps_t = ps_rot.tile([C, 2, 512], pdt, tag="rot", name="ps_t")
        ps_t = ps_t[:, 0, :]
        nc.tensor.transpose(ps_t[0:cnt, 0:C], vT_sb[:, m0:m0 + cnt],
                            identb[:])
        nc.vector.tensor_copy(
            v_sb[0:cnt, j, :, :],
            ps_t[0:cnt, 0:C].rearrange("p (h d) -> p h d", h=HEADS, d=HD))

    def exp_load(nch):
        """Prefetch the exp(rel) interleave for slot nch (4 parallel DMAs)."""
        e_sb = epool.tile([C, 7, HEADS, NSL], pdt, tag="expTI", name="e_sb")
        exp_of[nch] = e_sb
        flat = e_sb[:].rearrange("p a h n -> p (a h n)")
        tot = 7 * HEADS * NSL
        qtr = tot // 4
        for i in range(4):
            nc.sync.dma_start(out=flat[:, i * qtr:(i + 1) * qtr],
                              in_=expI_d[nch, :, i * qtr:(i + 1) * qtr])

    qT_sb = singles.tile([C, N], pdt)

    def prep_q(qc):
        """Whole-batch q projection, one 784-column piece at fill time."""
        ps_q = ps_rot.tile([C, 2, 512], f32, tag="rot", name="ps_q")
        for half in range(2):
            c0 = (2 * qc + half) * NSL
            nc.tensor.matmul(ps_q[:, half, 0:NSL], lhsT=wq_sb[:],
                             rhs=xT_sb[:, c0:c0 + NSL],
                             start=True, stop=True)
        nc.vector.tensor_scalar_add(qT_sb[:, 2 * qc * NSL:2 * (qc + 1) * NSL],
                                    ps_q[:, :, 0:NSL], bq_sb[:, 0:1])

    sco_of = {}

    def scores_round(g):
        """All 4 heads' score matmuls co-issued (4-way row packing) for
        global round g; emitted one round ahead of the exp that reads it,
        keeping score execution off the exp-to-exp critical chain."""
        if g >= NCH * 7:
            return
        nch, r = g // 7, g % 7
        tiles = []
        for hp in range(2):
            ps_s = ps_rot.tile([C, 2, 512], f32, tag="rot", name="ps_s")
            tiles.append(ps_s)
        for h in range(HEADS):
            nc.tensor.matmul(
                tiles[h // 2][0:128, h % 2, 0:NSL],
                lhsT=kT_sb[HD * h:HD * (h + 1), 128 * r:128 * (r + 1)],
                rhs=qT_sb[HD * h:HD * (h + 1), nch * NSL:(nch + 1) * NSL],
                start=True, stop=True,
                tile_position=(HD * h, 0))
        sco_of[g] = tiles

    def exp_mul(nch, r, hp):
        """exp + expR multiply for chunk r, head pair hp."""
        g = nch * 7 + r
        ps_s = sco_of[g][hp]
        if hp == 1:
            del sco_of[g]
        pt_sb = ptpool.tile([C, 2, NSL], pdt, tag="pt")
        nc.scalar.activation(pt_sb[:], ps_s[:, :, 0:NSL], AF.Exp)
        eng = nc.gpsimd if (r, hp) in GPS_MULS else nc.vector
        eng.tensor_mul(pp_of[nch][:, r, 2 * hp:2 * hp + 2, :], pt_sb[:],
                       exp_of[nch][:, r, 2 * hp:2 * hp + 2, :])

    def attnv4(nch, r):
        """attn@v + rowsums for kv chunk r: all 4 heads column-packed."""
        s = state[nch]
        m0, cnt = M_CHUNKS[r]
        if r == 0:
            s["ov"] = ps_out.tile([C, 512], f32, tag="out", name="ps_ov")
            s["z"] = ps_rs.tile([C, 512], f32, tag="rs", name="ps_z")
        ps_ov, ps_z = s["ov"], s["z"]
        pp = pp_of[nch]
        for h in range(HEADS):
            nc.tensor.matmul(
                ps_ov[HD * h:HD * (h + 1), 0:NSL],
                lhsT=v_sb[0:cnt, r, h, :],
                rhs=pp[0:cnt, r, h, :],
                start=(r == 0), stop=(r == len(M_CHUNKS) - 1),
                tile_position=(0, HD * h), skip_group_check=True)
        for h in range(HEADS):
            nc.tensor.matmul(
                ps_z[HD * h:HD * (h + 1), 0:NSL],
                lhsT=ones_sb[0:cnt, :],
                rhs=pp[0:cnt, r, h, :],
                start=(r == 0), stop=(r == len(M_CHUNKS) - 1),
                tile_position=(0, HD * h), skip_group_check=True)

    def extract(nch):
        """Normalize straight out of PSUM: recip(rowsums), multiply."""
        s = state[nch]
        ps_ov = s.pop("ov")
        ps_z = s.pop("z")
        rb_sb = opool.tile([C, NSL], f32, tag="rb")
        nc.vector.reciprocal_approx_fast(rb_sb[:], ps_z[0:C, 0:NSL])
        outT_sb = opool.tile([C, NSL], pdt, tag="outT")
        s["outT"] = outT_sb
        nc.vector.tensor_mul(outT_sb[:], ps_ov[0:C, 0:NSL], rb_sb[:])

    def proj_tail(nch):
        """Final projection in transposed layout; host untransposes."""
        s = state[nch]
        ps_ft = ps_rot.tile([C, 2, 512], f32, tag="rot", name="ps_ft")
        ps_ft = ps_ft[:, 0, :]
        nc.tensor.matmul(ps_ft[0:C, 0:NSL], lhsT=wp_sb[:],
                         rhs=s.pop("outT")[:], start=True, stop=True)
        fin_sb = opool.tile([C, NSL], f32, tag="fin")
        nc.vector.tensor_scalar_add(fin_sb[:], ps_ft[0:C, 0:NSL],
                                    bp_sb[:, 0:1])
        nc.sync.dma_start(out=out_d[:, nch * NSL:(nch + 1) * NSL],
                          in_=fin_sb[:])
        state.pop(nch)
        pp_of.pop(nch, None)
        exp_of.pop(nch, None)

    # ---- fill: k/v + all of q once, first exp tables ----
    exp_load(0)
    for w in range(2):
        for mc in range(2):
            prep_tap(w, mc)
    exp_load(1)
    for qc in range(4):
        prep_q(qc)
    for j in range(7):
        prep_vtrans(j)
    scores_round(0)
    # ---- steady loop over n-chunk slots ----
    for nch in range(NCH):
        pp_of[nch] = ppool.tile([C, 7, HEADS, NSL], pdt, tag="pp",
                                name="pp_sb")
        state.setdefault(nch, {})
        for step in range(14):
            r, hp = step // 2, step % 2
            if hp == 0:
                exp_mul(nch, r, hp)
                scores_round(nch * 7 + r + 1)
            else:
                exp_mul(nch, r, hp)
            if nch >= 1:
                if step % 2 == 0 and step <= 12:
                    attnv4(nch - 1, step // 2)
                elif step == 13:
                    extract(nch - 1)
            if nch >= 2 and step == 0:
                proj_tail(nch - 2)
            if step == 1 and nch + 2 < NCH:
                exp_load(nch + 2)
    # drain
    proj_tail(NCH - 2)
    for r in range(7):
        attnv4(NCH - 1, r)
    extract(NCH - 1)
    proj_tail(NCH - 1)


def _get_compiled():
    global _COMPILED
    if _COMPILED is None:
        _COMPILED = _build()
    return _COMPILED


def make_in_map(prep, j):
    return {
        "xT": np.ascontiguousarray(prep["xT"][j]),
        "expI": prep["expI"].reshape(NCH, C, 7 * HEADS * NSL),
        "Wq": prep["Wq"], "bq": prep["bq"],
        "Wktap": prep["Wk_tap"], "Wvtap": prep["Wv_tap"],
        "Wp": prep["Wp"], "bp": prep["bp"],
    }


def kernel(x, relative_pos, Wq, bq, Wk, bk, Wv, bv, conv_w, conv_b,
           bn_gamma, bn_beta, bn_mean, bn_var, Wp, bp, H=56, W=56,
           _trace=False):
    from concourse.bass_utils import run_bass_kernel_spmd

    prep = _host_prep(x, relative_pos, Wq, bq, Wk, bk, Wv, bv, conv_w,
                      conv_b, bn_gamma, bn_beta, bn_mean, bn_var, Wp, bp)
    nc = _get_compiled()

    in_maps = [make_in_map(prep, j) for j in range(NCORES)]

    res = run_bass_kernel_spmd(nc, in_maps, core_ids=list(range(NCORES)),
                               trace=_trace)

    out = np.empty((B, N, C), np.float32)
    for j in range(NCORES):
        out[j] = res.results[j]["out"].T
    if _trace:
        kernel._last_result = res
    return out
